# revision 49
# baseline (speedup 1.0000x reference)
"""HNHNConv Trainium2 kernel: 8-core SPMD Bass/Tile implementation.

Transfer-optimized for the ~35MB/s axon tunnel (wall time is transfer-
bound; device exec sits under the ~70ms axon dispatch floor):
  - x uploaded SHARDED (node round-robin by degree) as per-row int8 with
    an f16 scale packed into 2 extra byte columns; dequantized on-device
    to fp16 and AllGathered into a shared x_full table.
  - y quantized on-device to per-row uint8 (max-reduce -> reciprocal ->
    scaled ACT), f16 scale packed into 2 extra byte columns; dequantized
    host-side in the threaded per-shard fetch.
  - gather index tensors uploaded in their native 16-partition wrap
    ([16, cols]) and replicated to 128 partitions on-device.
  - pad gather entries point at dedicated zero rows (unused slots), so no
    rank-1 pad-correction matmuls and no alpha uploads.
  - custom cached PJRT runner: jitted executable + device-resident
    constants persist across calls; donated output buffers come from
    on-device jnp.zeros (first call) or the previous call's output.

Per core pipeline:
  dequant x_shard (int8 -> fp16), AllGather -> x_full
  B: dma_gather x_full rows (fp16, lo/hi int16 split, pads->zero rows)
     -> DVE segmented reduce -> ACT 1/cnt scale -> PE transpose
     -> W1 matmul -> ACT relu+b1 -> W2 matmul -> ACT +b2 (fp16)
     -> PE transpose -> e2 shard rows -> HBM
  AllGather e2 shards -> full e2 table (fp16)
  D: dma_gather e2 rows -> DVE segmented reduce -> ACT relu * 1/cnt
     -> per-row uint8 quantize -> y rows (u8 payload + f16 scale)
"""
import sys
sys.path.insert(0, "/opt/trn_rl_repo")
import os
import numpy as np

N_NODES, N_EDGES, N_INC, C = 50000, 25000, 600000, 128
NCORES = 8
ESLOTS, ET = 3200, 25
VSLOTS, VT = 6272, 49
LO = 32768
DCH = 4  # node tiles per phase-D gather call
GV_TOTAL = NCORES * VSLOTS          # 50176 rows in x_full
PAD_LO = 6250                       # zero row in lo half (core0, slot 6250)
PAD_HI = 7 * VSLOTS + 6250 - LO     # 17386: zero row in hi half (core7, slot 6250)

_cache = {}
LAST_EXEC_NS = None


def _prep(hyperedge_index):
    node = np.asarray(hyperedge_index[0]).astype(np.int64)
    edge = np.asarray(hyperedge_index[1]).astype(np.int64)
    cnt_e = np.bincount(edge, minlength=N_EDGES)
    cnt_v = np.bincount(node, minlength=N_NODES)

    # node -> (core, slot), round-robin by degree
    order_v = np.argsort(-cnt_v, kind="stable")
    core_of_node = np.empty(N_NODES, np.int64)
    slot_of_node = np.empty(N_NODES, np.int64)
    rv = np.arange(N_NODES)
    core_of_node[order_v] = rv % NCORES
    slot_of_node[order_v] = rv // NCORES
    g_v = core_of_node * VSLOTS + slot_of_node
    vtile = slot_of_node // 128
    Lv = np.zeros(VT, np.int64)
    np.maximum.at(Lv, vtile, cnt_v)

    # edge -> (core, slot); lo/hi split on the x_full row id
    lo_mask = g_v[node] < LO
    cnt_lo = np.bincount(edge[lo_mask], minlength=N_EDGES)
    cnt_hi = cnt_e - cnt_lo
    order_e = np.lexsort((-cnt_hi, -cnt_lo))
    for g in range(0, N_EDGES, 2048):
        seg = order_e[g:g + 2048]
        order_e[g:g + 2048] = seg[np.argsort(-cnt_hi[seg], kind="stable")]
    core_of_edge = np.empty(N_EDGES, np.int64)
    slot_of_edge = np.empty(N_EDGES, np.int64)
    r = np.arange(N_EDGES)
    core_of_edge[order_e] = r % NCORES
    slot_of_edge[order_e] = r // NCORES
    etile = slot_of_edge // 128
    Llo = np.zeros(ET, np.int64); Lhi = np.zeros(ET, np.int64)
    np.maximum.at(Llo, etile, cnt_lo)
    np.maximum.at(Lhi, etile, cnt_hi)

    inc_core = core_of_edge[edge]
    inc_slot = slot_of_edge[edge]
    side = (~lo_mask).astype(np.int64)
    key = edge * 2 + side
    oi = np.argsort(key, kind="stable")
    ks = key[oi]
    gs = np.r_[0, np.flatnonzero(np.diff(ks)) + 1]
    lays = np.arange(N_INC) - np.repeat(gs, np.diff(np.r_[gs, N_INC]))
    layer = np.empty(N_INC, np.int64)
    layer[oi] = lays
    idx_val = np.where(lo_mask, g_v[node], g_v[node] - LO).astype(np.int64)

    callsB = []
    off = 0
    for t in range(ET):
        for s, L in ((0, int(Llo[t])), (1, int(Lhi[t]))):
            if L == 0:
                continue
            callsB.append((t, s, L, off))
            off += L * 8
    CB = off
    idxB = np.empty((NCORES, 16, CB), np.int16)
    for (t, s, L, co) in callsB:
        idxB[:, :, co:co + L * 8] = PAD_LO if s == 0 else PAD_HI
    colB = {(cb[0], cb[1]): cb[3] for cb in callsB}
    j_in_call = layer * 128 + (inc_slot % 128)
    baseB = np.array([colB[(int(t), int(s))] for t, s in
                      zip(inc_slot // 128, side)])
    colsB = baseB + j_in_call // 16
    for c in range(NCORES):
        m = inc_core == c
        idxB[c, j_in_call[m] % 16, colsB[m]] = idx_val[m].astype(np.int16)

    cnt_slot = np.zeros((NCORES, ESLOTS), np.int64)
    cnt_slot[core_of_edge, slot_of_edge] = cnt_e
    recip_e = (1.0 / np.maximum(cnt_slot, 1)).astype(np.float32)

    e2row = (core_of_edge * ESLOTS + slot_of_edge).astype(np.int64)
    ZROW = NCORES * ESLOTS
    oi2 = np.argsort(node, kind="stable")
    ns = node[oi2]
    g2 = np.r_[0, np.flatnonzero(np.diff(ns)) + 1]
    lay2 = np.arange(N_INC) - np.repeat(g2, np.diff(np.r_[g2, N_INC]))
    layerD = np.empty(N_INC, np.int64)
    layerD[oi2] = lay2

    callsD = []
    tile_off = np.zeros(VT + 1, np.int64)
    offD = 0
    for t0 in range(0, VT, DCH):
        ts = list(range(t0, min(t0 + DCH, VT)))
        Ls = int(sum(Lv[t] for t in ts))
        callsD.append((t0, len(ts), Ls, offD))
        acc = 0
        for t in ts:
            tile_off[t] = acc
            acc += int(Lv[t])
        offD += Ls * 8
    CD = offD
    idxD = np.full((NCORES, 16, CD), ZROW, np.int16)
    inc_vcore = core_of_node[node]
    inc_vslot = slot_of_node[node]
    t_v = inc_vslot // 128
    call_of_tile = np.zeros(VT, np.int64)
    for ci, (t0, nt, Ls, co) in enumerate(callsD):
        call_of_tile[t0:t0 + nt] = ci
    callD_col = np.array([callsD[int(ci)][3] for ci in call_of_tile[t_v]])
    jD = (tile_off[t_v] + layerD) * 128 + (inc_vslot % 128)
    colsD = callD_col + jD // 16
    for c in range(NCORES):
        m = inc_vcore == c
        idxD[c, jD[m] % 16, colsD[m]] = e2row[edge[m]].astype(np.int16)

    cnt_vslot = np.zeros((NCORES, VSLOTS), np.int64)
    cnt_vslot[core_of_node, slot_of_node] = cnt_v
    recip_v = (1.0 / np.maximum(cnt_vslot, 1)).astype(np.float32)

    return dict(Llo=Llo, Lhi=Lhi, Lv=Lv, callsB=callsB, callsD=callsD,
                CB=CB, CD=CD, idxB=idxB, idxD=idxD,
                recip_e=recip_e, recip_v=recip_v,
                g_v=g_v, tile_off=tile_off, ZROW=ZROW)


def _build(P):
    import concourse.bass as bass
    import concourse.mybir as mybir
    import concourse.tile as tile
    from concourse import bacc

    f32, f16, i16 = mybir.dt.float32, mybir.dt.float16, mybir.dt.int16
    u8, i8 = mybir.dt.uint8, mybir.dt.int8
    Relu = mybir.ActivationFunctionType.Relu
    Ident = mybir.ActivationFunctionType.Identity
    Copy = mybir.ActivationFunctionType.Copy
    AddOp = mybir.AluOpType.add
    MaxOp = mybir.AluOpType.max
    MultOp = mybir.AluOpType.mult
    AX = mybir.AxisListType.X

    Llo, Lhi, Lv = P["Llo"], P["Lhi"], P["Lv"]
    CB, CD = P["CB"], P["CD"]
    KPH = os.environ.get("HNHN_DEBUG_PHASES", "XBCD")  # debug bisection only

    nc = bacc.Bacc("TRN2", target_bir_lowering=False, debug=False,
                   num_devices=NCORES)

    # x uploaded as per-row int8 + f16 scale packed into byte columns
    xs_t = nc.dram_tensor("x_shard", [VSLOTS, C + 2], u8, kind="ExternalInput")
    idxB_t = nc.dram_tensor("idxB", [16, CB], i16, kind="ExternalInput")
    idxD_t = nc.dram_tensor("idxD", [16, CD], i16, kind="ExternalInput")
    re_t = nc.dram_tensor("recip_e", [128, ET], f32, kind="ExternalInput")
    rv_t = nc.dram_tensor("recip_v", [128, VT], f32, kind="ExternalInput")
    w1t_t = nc.dram_tensor("w1t", [C, C], f32, kind="ExternalInput")
    w2t_t = nc.dram_tensor("w2t", [C, C], f32, kind="ExternalInput")
    b1_t = nc.dram_tensor("b1", [C, 1], f32, kind="ExternalInput")
    b2_t = nc.dram_tensor("b2", [C, 1], f32, kind="ExternalInput")
    eye32_t = nc.dram_tensor("eye32", [C, C], f32, kind="ExternalInput")
    eyeH_t = nc.dram_tensor("eyeH", [C, C], f16, kind="ExternalInput")
    # y quantized to uint8 with a per-row scale; scale (f16) packed into the
    # last two byte-columns so the host fetches one tensor per shard
    y_t = nc.dram_tensor("y", [VSLOTS, C + 2], u8, kind="ExternalOutput")

    x_stage = nc.dram_tensor("x_stage", [VSLOTS, C], f16)
    x_full = nc.dram_tensor("x_full", [GV_TOTAL, C], f16, addr_space="Shared")
    e2_shard = nc.dram_tensor("e2_shard", [ESLOTS, C], f16)
    e2_table = nc.dram_tensor("e2_table", [NCORES * ESLOTS + 128, C], f16,
                              addr_space="Shared")

    with tile.TileContext(nc) as tc:
        with (
            tc.tile_pool(name="const", bufs=1) as cpool,
            tc.tile_pool(name="idx", bufs=1) as ipool,
            tc.tile_pool(name="strip", bufs=3) as spool,
            tc.tile_pool(name="work", bufs=3) as wpool,
            tc.tile_pool(name="psA", bufs=1, space="PSUM") as psA,
            tc.tile_pool(name="psB", bufs=2, space="PSUM") as psB,
        ):
            # ---- constant uploads
            w1t = cpool.tile([C, C], f32, tag="w1t")
            w2t = cpool.tile([C, C], f32, tag="w2t")
            b1 = cpool.tile([C, 1], f32, tag="b1")
            b2 = cpool.tile([C, 1], f32, tag="b2")
            eye32 = cpool.tile([C, C], f32, tag="eye32")
            eyeH = cpool.tile([C, C], f16, tag="eyeH")
            re = cpool.tile([128, ET], f32, tag="re")
            rv = cpool.tile([128, VT], f32, tag="rv")
            idxB = ipool.tile([128, CB], i16, tag="idxB")
            idxD = ipool.tile([128, CD], i16, tag="idxD")
            zrow = cpool.tile([1, C], f16, tag="zrow")
            half = cpool.tile([128, 1], f32, tag="half")
            nc.vector.memset(half[:, :], 0.5)

            nc.sync.dma_start(w1t[:, :], w1t_t[:, :])
            nc.sync.dma_start(w2t[:, :], w2t_t[:, :])
            nc.sync.dma_start(b1[:, :], b1_t[:, :])
            nc.sync.dma_start(b2[:, :], b2_t[:, :])
            nc.sync.dma_start(eye32[:, :], eye32_t[:, :])
            nc.sync.dma_start(eyeH[:, :], eyeH_t[:, :])
            nc.sync.dma_start(re[:, :], re_t[:, :])
            nc.sync.dma_start(rv[:, :], rv_t[:, :])
            # replicate the 16-partition wrapped idx patterns to 128
            for k in range(8):
                nc.sync.dma_start(idxB[16 * k:16 * (k + 1), :], idxB_t[:, :])
                nc.sync.dma_start(idxD[16 * k:16 * (k + 1), :], idxD_t[:, :])
            nc.vector.memset(zrow[:, :], 0.0)
            nc.sync.dma_start(e2_table[P["ZROW"]:P["ZROW"] + 1, :], zrow[:, :])

            # ---- dequantize x shard into the f16 stage, then AllGather
            # (stage: collectives can't read IO tensors directly)
            if "X" in KPH:
                for t in range(VT):
                    sl0 = slice(t * 128, (t + 1) * 128)
                    xq = wpool.tile([128, C + 2], u8, tag="xq")
                    nc.sync.dma_start(xq[:, :], xs_t[sl0, :])
                    xscf = wpool.tile([128, 1], f32, tag="xscf")
                    nc.scalar.copy(xscf[:, :], xq[:, C:C + 2].bitcast(f16))
                    xd = wpool.tile([128, C], f16, tag="xd")
                    nc.scalar.activation(xd[:, :], xq[:, 0:C].bitcast(i8),
                                         Ident, bias=0.0,
                                         scale=xscf[:, 0:1])
                    nc.sync.dma_start(x_stage[sl0, :], xd[:, :])
                nc.gpsimd.collective_compute(
                    "AllGather", mybir.AluOpType.bypass,
                    replica_groups=[list(range(NCORES))],
                    ins=[x_stage.ap().opt()],
                    outs=[x_full[0:GV_TOTAL, :].opt()])

            callB_of_tile = {}
            for (t, s, L, co) in P["callsB"]:
                callB_of_tile.setdefault(t, []).append((s, L, co))

            # ---- phase B per edge tile
            for t in (range(ET) if "B" in KPH else []):
                Lt = int(Llo[t] + Lhi[t])
                strip = spool.tile([128, Lt, C], f16, tag="strip")
                loff = 0
                for (s, L, co) in callB_of_tile[t]:
                    src = x_full[0:LO, :] if s == 0 else x_full[LO:GV_TOTAL, :]
                    nc.gpsimd.dma_gather(
                        strip[:, loff:loff + L, :], src,
                        idxB[:, co:co + L * 8], L * 128, L * 128, C,
                        single_packet=False)
                    loff += L
                sl = slice(t * 128, (t + 1) * 128)
                xsum = wpool.tile([128, C], f32, tag="xsum")
                nc.vector.tensor_reduce(
                    xsum[:, :], strip[:, :, :].rearrange("p l f -> p f l"),
                    AX, AddOp)
                xm = wpool.tile([128, C], f32, tag="xm")
                nc.scalar.activation(xm[:, :], xsum[:, :], Copy,
                                     bias=0.0, scale=re[:, t:t + 1])
                # transpose -> [feat, slot]
                pT = psA.tile([128, C], f32, tag="pT")
                nc.tensor.transpose(pT[:, :], xm[:, :], eye32[:, :])
                xmT = wpool.tile([128, C], f32, tag="xmT")
                nc.scalar.copy(xmT[:, :], pT[:, :])
                # W1 -> relu(+b1)
                pe = psB.tile([128, C], f32, tag="pe")
                nc.tensor.matmul(pe[:, :], w1t[:, :], xmT[:, :])
                eT = wpool.tile([128, C], f32, tag="eT")
                nc.scalar.activation(eT[:, :], pe[:, :], Relu,
                                     bias=b1[:, :], scale=1.0)
                # W2 -> +b2 (fp16)
                pe2 = psB.tile([128, C], f32, tag="pe2")
                nc.tensor.matmul(pe2[:, :], w2t[:, :], eT[:, :])
                e2T = wpool.tile([128, C], f16, tag="e2T")
                nc.scalar.activation(e2T[:, :], pe2[:, :], Ident,
                                     bias=b2[:, :], scale=1.0)
                # transpose back -> e2 rows, store shard
                pr = psA.tile([128, C], f16, tag="pr")
                nc.tensor.transpose(pr[:, :], e2T[:, :], eyeH[:, :])
                e2r = wpool.tile([128, C], f16, tag="e2r")
                nc.scalar.copy(e2r[:, :], pr[:, :])
                nc.sync.dma_start(e2_shard[sl, :], e2r[:, :])

            # ---- AllGather e2 shards
            if "C" in KPH:
                nc.gpsimd.collective_compute(
                    "AllGather", mybir.AluOpType.bypass,
                    replica_groups=[list(range(NCORES))],
                    ins=[e2_shard.ap().opt()],
                    outs=[e2_table[0:NCORES * ESLOTS, :].opt()])

            # ---- phase D
            for (t0, nt, Ls, co) in (P["callsD"] if "D" in KPH else []):
                dstrip = spool.tile([128, Ls, C], f16, tag="dstrip")
                nc.gpsimd.dma_gather(
                    dstrip[:, :, :], e2_table[:, :],
                    idxD[:, co:co + Ls * 8], Ls * 128, Ls * 128, C,
                    single_packet=False)
                for t in range(t0, t0 + nt):
                    L = int(Lv[t])
                    toff = int(P["tile_off"][t])
                    ysum = wpool.tile([128, C], f32, tag="ysum")
                    nc.vector.tensor_reduce(
                        ysum[:, :],
                        dstrip[:, toff:toff + L, :].rearrange("p l f -> p f l"),
                        AX, AddOp)
                    yt = wpool.tile([128, C], f32, tag="yt")
                    nc.scalar.activation(yt[:, :], ysum[:, :], Relu,
                                         bias=0.0, scale=rv[:, t:t + 1])
                    # per-row uint8 quantization: q = round(y * 254/rowmax)
                    rmax = wpool.tile([128, 1], f32, tag="rmax")
                    nc.vector.tensor_reduce(rmax[:, :], yt[:, :], AX, MaxOp)
                    # rmax <- max(rmax, eps) / 254   (the stored scale)
                    nc.vector.tensor_scalar(rmax[:, :], rmax[:, :], 1e-20,
                                            1.0 / 254.0, MaxOp, MultOp)
                    sinv = wpool.tile([128, 1], f32, tag="sinv")
                    nc.vector.reciprocal(sinv[:, :], rmax[:, :])
                    yq = wpool.tile([128, C], u8, tag="yq")
                    nc.scalar.activation(yq[:, :], yt[:, :], Ident,
                                         bias=half[:, :], scale=sinv[:, 0:1])
                    ysc = wpool.tile([128, 1], f16, tag="ysc")
                    nc.scalar.copy(ysc[:, :], rmax[:, :])
                    sl2 = slice(t * 128, (t + 1) * 128)
                    nc.sync.dma_start(y_t[sl2, 0:C], yq[:, :])
                    nc.sync.dma_start(y_t[sl2, C:C + 2],
                                      ysc[:, :].bitcast(u8))
    nc.compile()
    return nc


def _get_runner(nc):
    import jax
    import jax.numpy as jnp
    import concourse.mybir as mybir
    from concourse.bass2jax import (_bass_exec_p, install_neuronx_cc_hook,
                                    partition_id_tensor)
    from jax.sharding import Mesh, PartitionSpec, NamedSharding
    from jax.experimental.shard_map import shard_map

    install_neuronx_cc_hook()
    partition_name = (nc.partition_id_tensor.name
                      if nc.partition_id_tensor else None)
    in_names, out_names, out_avals = [], [], []
    for alloc in nc.m.functions[0].allocations:
        if not isinstance(alloc, mybir.MemoryLocationSet):
            continue
        name = alloc.memorylocations[0].name
        if alloc.kind == "ExternalInput":
            if name != partition_name:
                in_names.append(name)
        elif alloc.kind == "ExternalOutput":
            out_names.append(name)
            out_avals.append(jax.core.ShapedArray(
                tuple(alloc.tensor_shape), mybir.dt.np(alloc.dtype)))
    n_params = len(in_names)
    n_outs = len(out_names)
    all_names = in_names + out_names + (
        [partition_name] if partition_name else [])

    def _body(*args):
        operands = list(args)
        if partition_name is not None:
            operands.append(partition_id_tensor())
        outs = _bass_exec_p.bind(
            *operands, out_avals=tuple(out_avals),
            in_names=tuple(all_names), out_names=tuple(out_names),
            lowering_input_output_aliases=(), sim_require_finite=True,
            sim_require_nnan=True, nc=nc)
        return tuple(outs)

    devices = jax.devices()[:NCORES]
    mesh = Mesh(np.asarray(devices), ("core",))
    spec = PartitionSpec("core")
    in_specs = (spec,) * (n_params + n_outs)
    out_specs = (spec,) * n_outs
    donate = tuple(range(n_params, n_params + n_outs))
    fn = jax.jit(
        shard_map(_body, mesh=mesh, in_specs=in_specs,
                  out_specs=out_specs, check_rep=False),
        donate_argnums=donate, keep_unused=True)
    sh = NamedSharding(mesh, spec)
    zfns = [jax.jit(
        lambda a=av: jnp.zeros((NCORES * a.shape[0],) + a.shape[1:], a.dtype),
        out_shardings=sh) for av in out_avals]
    return dict(fn=fn, in_names=in_names, out_names=out_names,
                sh=sh, zfns=zfns, devices=devices)


def kernel(x, hyperedge_index, W_v2e, b_v2e, W_e2v, b_e2v):
    import jax
    import time
    KTIME = os.environ.get("HNHN_DEBUG_TIME", "0") == "1"
    tick = time.time

    t0 = tick()
    hb = np.asarray(hyperedge_index)
    cached_hb = _cache.get("hb")
    if (cached_hb is None or cached_hb.shape != hb.shape
            or cached_hb.dtype != hb.dtype or not np.array_equal(cached_hb, hb)):
        _cache.clear()
        _cache["hb"] = hb.copy()
        _cache["P"] = _prep(hb)
        _cache["nc"] = _build(_cache["P"])
        _cache["R"] = _get_runner(_cache["nc"])
        _cache["dev"] = {}
    P, R = _cache["P"], _cache["R"]
    dev = _cache["dev"]
    sh = R["sh"]
    pool = _cache.get("pool")
    if pool is None:
        from concurrent.futures import ThreadPoolExecutor
        pool = _cache["pool"] = ThreadPoolExecutor(NCORES)

    # per-row int8 quantization (f16 scale packed in the last 2 columns),
    # threaded across row chunks, then one async device_put
    txs = tick()
    xs = _cache.get("xs")
    if xs is None:
        xs = _cache["xs"] = np.zeros((GV_TOTAL, C + 2), np.uint8)
    xf = np.asarray(x, np.float32)
    g_v = P["g_v"]

    def _quant(a):
        b = min(a + 6250, N_NODES)
        xc = xf[a:b]
        sc = (np.maximum(np.abs(xc).max(axis=1), 1e-20) / 127.0
              ).astype(np.float16)
        tmp = xc * (1.0 / sc.astype(np.float32))[:, None]
        np.rint(tmp, out=tmp)
        np.clip(tmp, -127, 127, out=tmp)
        gv = g_v[a:b]
        xs[gv, 0:C] = tmp.astype(np.int8).view(np.uint8)
        xs[gv, C:C + 2] = sc.view(np.uint8).reshape(-1, 2)
    list(pool.map(_quant, range(0, N_NODES, 6250)))
    if KTIME: print("  xs scatter:", tick() - txs)
    dx = jax.device_put(xs, sh)
    if KTIME:
        _tu = tick(); jax.block_until_ready(dx)
        print("  x upload wait:", tick() - _tu)

    def put(name, arr):
        cur = dev.get(name)
        if cur is None or not (cur[0] is arr or np.array_equal(cur[0], arr)):
            dev[name] = (arr, jax.device_put(arr, sh))
        return dev[name][1]

    if "const_np" not in _cache:
        CB, CD = P["CB"], P["CD"]
        _cache["const_np"] = {
            "idxB": np.ascontiguousarray(P["idxB"].reshape(NCORES * 16, CB)),
            "idxD": np.ascontiguousarray(P["idxD"].reshape(NCORES * 16, CD)),
            "recip_e": np.ascontiguousarray(
                P["recip_e"].reshape(NCORES, ET, 128).transpose(0, 2, 1)
            ).reshape(NCORES * 128, ET),
            "recip_v": np.ascontiguousarray(
                P["recip_v"].reshape(NCORES, VT, 128).transpose(0, 2, 1)
            ).reshape(NCORES * 128, VT),
            "eye32": np.tile(np.eye(C, dtype=np.float32), (NCORES, 1)),
            "eyeH": np.tile(np.eye(C, dtype=np.float16), (NCORES, 1)),
        }
    cn = _cache["const_np"]

    # weights: compare the small untiled arrays, cache tiled device copies
    def putw(name, arr):
        cur = dev.get(name)
        if cur is None or not (cur[0] is arr or np.array_equal(cur[0], arr)):
            tiled = np.tile(np.ascontiguousarray(arr), (NCORES, 1))
            dev[name] = (arr, jax.device_put(tiled, sh))
        return dev[name][1]

    w1t = np.asarray(W_v2e, np.float32).T
    w2t = np.asarray(W_e2v, np.float32).T
    b1 = np.asarray(b_v2e, np.float32).reshape(C, 1)
    b2 = np.asarray(b_e2v, np.float32).reshape(C, 1)

    named = {"idxB": cn["idxB"], "idxD": cn["idxD"],
             "recip_e": cn["recip_e"], "recip_v": cn["recip_v"],
             "eye32": cn["eye32"], "eyeH": cn["eyeH"]}
    wnamed = {"w1t": w1t, "w2t": w2t, "b1": b1, "b2": b2}
    args = []
    for name in R["in_names"]:
        if name == "x_shard":
            args.append(dx)
        elif name in wnamed:
            args.append(putw(name, wnamed[name]))
        else:
            args.append(put(name, named[name]))
    # donate the previous call's output buffer when available (the kernel
    # writes every row of y, so initial contents are irrelevant)
    zeros = _cache.pop("donate_next", None)
    if zeros is None:
        zeros = [zf() for zf in R["zfns"]]
    if KTIME:
        jax.block_until_ready(args); jax.block_until_ready(zeros)
        print("  consts+zeros+xwait:", tick() - t0)
        t0 = tick()
    outs = R["fn"](*args, *zeros)
    if KTIME:
        jax.block_until_ready(outs)
        print("  exec:", tick() - t0)
        t0 = tick()
    yi = R["out_names"].index("y")
    out = np.empty((N_NODES, C), np.float32)
    if "inv" not in _cache:
        # per-core: node ids owned by that core (ordered) and their slots
        g_v = P["g_v"]
        core = g_v // VSLOTS
        _cache["inv"] = [(np.flatnonzero(core == c),
                          g_v[core == c] - c * VSLOTS)
                         for c in range(NCORES)]
    inv = _cache["inv"]
    shards = outs[yi].addressable_shards

    def _fetch(s):
        c = s.index[0].start // VSLOTS
        ys = np.asarray(s.data)  # [VSLOTS, C+2] uint8
        nodes, slots = inv[c]
        sc = ys[slots, C:C + 2].copy().view(np.float16).astype(np.float32)
        out[nodes] = ys[slots, 0:C] * sc
    list(pool.map(_fetch, shards))
    _cache["donate_next"] = list(outs)
    if KTIME: print("  y fetch+scatter:", tick() - t0)
    return out


# revision 53
# speedup vs baseline: 1.1094x; 1.1094x over previous
"""HNHNConv Trainium2 kernel: 8-core SPMD Bass/Tile implementation.

Transfer-optimized for the ~35MB/s axon tunnel (wall time is transfer-
bound; device exec sits under the ~70ms axon dispatch floor):
  - x uploaded SHARDED (node round-robin by degree) as per-row int8 with
    an f16 scale packed into 2 extra byte columns; dequantized on-device
    to fp16 and AllGathered into a shared x_full table.
  - y quantized on-device to per-row uint8 (max-reduce -> reciprocal ->
    scaled ACT), f16 scale packed into 2 extra byte columns; dequantized
    host-side in the threaded per-shard fetch.
  - gather index tensors uploaded in their native 16-partition wrap
    ([16, cols]) and replicated to 128 partitions on-device.
  - pad gather entries point at dedicated zero rows (unused slots), so no
    rank-1 pad-correction matmuls and no alpha uploads.
  - custom cached PJRT runner: jitted executable + device-resident
    constants persist across calls; donated output buffers come from
    on-device jnp.zeros (first call) or the previous call's output.

Per core pipeline:
  dequant x_shard (int8 -> fp16), AllGather -> x_full
  B: dma_gather x_full rows (fp16, lo/hi int16 split, pads->zero rows)
     -> DVE segmented reduce -> ACT 1/cnt scale -> PE transpose
     -> W1 matmul -> ACT relu+b1 -> W2 matmul -> ACT +b2 (fp16)
     -> PE transpose -> e2 shard rows -> HBM
  AllGather e2 shards -> full e2 table (fp16)
  D: dma_gather e2 rows -> DVE segmented reduce -> ACT relu * 1/cnt
     -> per-row uint8 quantize -> y rows (u8 payload + f16 scale)
"""
import sys
sys.path.insert(0, "/opt/trn_rl_repo")
import os
import numpy as np

N_NODES, N_EDGES, N_INC, C = 50000, 25000, 600000, 128
NCORES = 8
ESLOTS, ET = 3200, 25
VSLOTS, VT = 6272, 49
LO = 32768
DCH = 4  # node tiles per phase-D gather call
GV_TOTAL = NCORES * VSLOTS          # 50176 rows in x_full
PAD_LO = 6250                       # zero row in lo half (core0, slot 6250)
PAD_HI = 7 * VSLOTS + 6250 - LO     # 17386: zero row in hi half (core7, slot 6250)

_cache = {}
LAST_EXEC_NS = None


def _prep(hyperedge_index):
    node = np.asarray(hyperedge_index[0]).astype(np.int64)
    edge = np.asarray(hyperedge_index[1]).astype(np.int64)
    cnt_e = np.bincount(edge, minlength=N_EDGES)
    cnt_v = np.bincount(node, minlength=N_NODES)

    # node -> (core, slot), round-robin by degree
    order_v = np.argsort(-cnt_v, kind="stable")
    core_of_node = np.empty(N_NODES, np.int64)
    slot_of_node = np.empty(N_NODES, np.int64)
    rv = np.arange(N_NODES)
    core_of_node[order_v] = rv % NCORES
    slot_of_node[order_v] = rv // NCORES
    g_v = core_of_node * VSLOTS + slot_of_node
    vtile = slot_of_node // 128
    Lv = np.zeros(VT, np.int64)
    np.maximum.at(Lv, vtile, cnt_v)

    # x_full table row: pad-aligned NODE order (not slot order) so the host
    # quantizer reads/writes contiguously; each core's shard is nodes
    # [6250c, 6250(c+1)) in rows [0, 6250) with rows [6250, 6272) zero pads
    npc = N_NODES // NCORES
    rx = np.arange(N_NODES) + (VSLOTS - npc) * (np.arange(N_NODES) // npc)

    # edge -> (core, slot); lo/hi split on the x_full row id
    lo_mask = rx[node] < LO
    cnt_lo = np.bincount(edge[lo_mask], minlength=N_EDGES)
    cnt_hi = cnt_e - cnt_lo
    order_e = np.lexsort((-cnt_hi, -cnt_lo))
    for g in range(0, N_EDGES, 2048):
        seg = order_e[g:g + 2048]
        order_e[g:g + 2048] = seg[np.argsort(-cnt_hi[seg], kind="stable")]
    core_of_edge = np.empty(N_EDGES, np.int64)
    slot_of_edge = np.empty(N_EDGES, np.int64)
    r = np.arange(N_EDGES)
    core_of_edge[order_e] = r % NCORES
    slot_of_edge[order_e] = r // NCORES
    etile = slot_of_edge // 128
    Llo = np.zeros(ET, np.int64); Lhi = np.zeros(ET, np.int64)
    np.maximum.at(Llo, etile, cnt_lo)
    np.maximum.at(Lhi, etile, cnt_hi)

    inc_core = core_of_edge[edge]
    inc_slot = slot_of_edge[edge]
    side = (~lo_mask).astype(np.int64)
    key = edge * 2 + side
    oi = np.argsort(key, kind="stable")
    ks = key[oi]
    gs = np.r_[0, np.flatnonzero(np.diff(ks)) + 1]
    lays = np.arange(N_INC) - np.repeat(gs, np.diff(np.r_[gs, N_INC]))
    layer = np.empty(N_INC, np.int64)
    layer[oi] = lays
    idx_val = np.where(lo_mask, rx[node], rx[node] - LO).astype(np.int64)

    callsB = []
    off = 0
    for t in range(ET):
        for s, L in ((0, int(Llo[t])), (1, int(Lhi[t]))):
            if L == 0:
                continue
            callsB.append((t, s, L, off))
            off += L * 8
    CB = off
    idxB = np.empty((NCORES, 16, CB), np.int16)
    for (t, s, L, co) in callsB:
        idxB[:, :, co:co + L * 8] = PAD_LO if s == 0 else PAD_HI
    colB = {(cb[0], cb[1]): cb[3] for cb in callsB}
    j_in_call = layer * 128 + (inc_slot % 128)
    baseB = np.array([colB[(int(t), int(s))] for t, s in
                      zip(inc_slot // 128, side)])
    colsB = baseB + j_in_call // 16
    for c in range(NCORES):
        m = inc_core == c
        idxB[c, j_in_call[m] % 16, colsB[m]] = idx_val[m].astype(np.int16)

    cnt_slot = np.zeros((NCORES, ESLOTS), np.int64)
    cnt_slot[core_of_edge, slot_of_edge] = cnt_e
    recip_e = (1.0 / np.maximum(cnt_slot, 1)).astype(np.float32)

    e2row = (core_of_edge * ESLOTS + slot_of_edge).astype(np.int64)
    ZROW = NCORES * ESLOTS
    oi2 = np.argsort(node, kind="stable")
    ns = node[oi2]
    g2 = np.r_[0, np.flatnonzero(np.diff(ns)) + 1]
    lay2 = np.arange(N_INC) - np.repeat(g2, np.diff(np.r_[g2, N_INC]))
    layerD = np.empty(N_INC, np.int64)
    layerD[oi2] = lay2

    callsD = []
    tile_off = np.zeros(VT + 1, np.int64)
    offD = 0
    for t0 in range(0, VT, DCH):
        ts = list(range(t0, min(t0 + DCH, VT)))
        Ls = int(sum(Lv[t] for t in ts))
        callsD.append((t0, len(ts), Ls, offD))
        acc = 0
        for t in ts:
            tile_off[t] = acc
            acc += int(Lv[t])
        offD += Ls * 8
    CD = offD
    idxD = np.full((NCORES, 16, CD), ZROW, np.int16)
    inc_vcore = core_of_node[node]
    inc_vslot = slot_of_node[node]
    t_v = inc_vslot // 128
    call_of_tile = np.zeros(VT, np.int64)
    for ci, (t0, nt, Ls, co) in enumerate(callsD):
        call_of_tile[t0:t0 + nt] = ci
    callD_col = np.array([callsD[int(ci)][3] for ci in call_of_tile[t_v]])
    jD = (tile_off[t_v] + layerD) * 128 + (inc_vslot % 128)
    colsD = callD_col + jD // 16
    for c in range(NCORES):
        m = inc_vcore == c
        idxD[c, jD[m] % 16, colsD[m]] = e2row[edge[m]].astype(np.int16)

    cnt_vslot = np.zeros((NCORES, VSLOTS), np.int64)
    cnt_vslot[core_of_node, slot_of_node] = cnt_v
    recip_v = (1.0 / np.maximum(cnt_vslot, 1)).astype(np.float32)

    return dict(Llo=Llo, Lhi=Lhi, Lv=Lv, callsB=callsB, callsD=callsD,
                CB=CB, CD=CD, idxB=idxB, idxD=idxD,
                recip_e=recip_e, recip_v=recip_v,
                g_v=g_v, tile_off=tile_off, ZROW=ZROW)


def _build(P):
    import concourse.bass as bass
    import concourse.mybir as mybir
    import concourse.tile as tile
    from concourse import bacc

    f32, f16, i16 = mybir.dt.float32, mybir.dt.float16, mybir.dt.int16
    u8, i8 = mybir.dt.uint8, mybir.dt.int8
    Relu = mybir.ActivationFunctionType.Relu
    Ident = mybir.ActivationFunctionType.Identity
    Copy = mybir.ActivationFunctionType.Copy
    AddOp = mybir.AluOpType.add
    MaxOp = mybir.AluOpType.max
    MultOp = mybir.AluOpType.mult
    AX = mybir.AxisListType.X

    Llo, Lhi, Lv = P["Llo"], P["Lhi"], P["Lv"]
    CB, CD = P["CB"], P["CD"]
    KPH = os.environ.get("HNHN_DEBUG_PHASES", "XBCD")  # debug bisection only

    nc = bacc.Bacc("TRN2", target_bir_lowering=False, debug=False,
                   num_devices=NCORES)

    # x uploaded as per-row int8 + f16 scale packed into byte columns
    xs_t = nc.dram_tensor("x_shard", [VSLOTS, C + 2], u8, kind="ExternalInput")
    idxB_t = nc.dram_tensor("idxB", [16, CB], i16, kind="ExternalInput")
    idxD_t = nc.dram_tensor("idxD", [16, CD], i16, kind="ExternalInput")
    re_t = nc.dram_tensor("recip_e", [128, ET], f32, kind="ExternalInput")
    rv_t = nc.dram_tensor("recip_v", [128, VT], f32, kind="ExternalInput")
    w1t_t = nc.dram_tensor("w1t", [C, C], f32, kind="ExternalInput")
    w2t_t = nc.dram_tensor("w2t", [C, C], f32, kind="ExternalInput")
    b1_t = nc.dram_tensor("b1", [C, 1], f32, kind="ExternalInput")
    b2_t = nc.dram_tensor("b2", [C, 1], f32, kind="ExternalInput")
    eye32_t = nc.dram_tensor("eye32", [C, C], f32, kind="ExternalInput")
    eyeH_t = nc.dram_tensor("eyeH", [C, C], f16, kind="ExternalInput")
    # y quantized to uint8 with a per-row scale; scale (f16) packed into the
    # last two byte-columns so the host fetches one tensor per shard
    y_t = nc.dram_tensor("y", [VSLOTS, C + 2], u8, kind="ExternalOutput")

    x_stage = nc.dram_tensor("x_stage", [VSLOTS, C], f16)
    x_full = nc.dram_tensor("x_full", [GV_TOTAL, C], f16, addr_space="Shared")
    e2_shard = nc.dram_tensor("e2_shard", [ESLOTS, C], f16)
    e2_table = nc.dram_tensor("e2_table", [NCORES * ESLOTS + 128, C], f16,
                              addr_space="Shared")

    with tile.TileContext(nc) as tc:
        with (
            tc.tile_pool(name="const", bufs=1) as cpool,
            tc.tile_pool(name="idx", bufs=1) as ipool,
            tc.tile_pool(name="strip", bufs=3) as spool,
            tc.tile_pool(name="work", bufs=3) as wpool,
            tc.tile_pool(name="psA", bufs=1, space="PSUM") as psA,
            tc.tile_pool(name="psB", bufs=2, space="PSUM") as psB,
        ):
            # ---- constant uploads
            w1t = cpool.tile([C, C], f32, tag="w1t")
            w2t = cpool.tile([C, C], f32, tag="w2t")
            b1 = cpool.tile([C, 1], f32, tag="b1")
            b2 = cpool.tile([C, 1], f32, tag="b2")
            eye32 = cpool.tile([C, C], f32, tag="eye32")
            eyeH = cpool.tile([C, C], f16, tag="eyeH")
            re = cpool.tile([128, ET], f32, tag="re")
            rv = cpool.tile([128, VT], f32, tag="rv")
            idxB = ipool.tile([128, CB], i16, tag="idxB")
            idxD = ipool.tile([128, CD], i16, tag="idxD")
            zrow = cpool.tile([1, C], f16, tag="zrow")
            half = cpool.tile([128, 1], f32, tag="half")
            nc.vector.memset(half[:, :], 0.5)

            nc.sync.dma_start(w1t[:, :], w1t_t[:, :])
            nc.sync.dma_start(w2t[:, :], w2t_t[:, :])
            nc.sync.dma_start(b1[:, :], b1_t[:, :])
            nc.sync.dma_start(b2[:, :], b2_t[:, :])
            nc.sync.dma_start(eye32[:, :], eye32_t[:, :])
            nc.sync.dma_start(eyeH[:, :], eyeH_t[:, :])
            nc.sync.dma_start(re[:, :], re_t[:, :])
            nc.sync.dma_start(rv[:, :], rv_t[:, :])
            # replicate the 16-partition wrapped idx patterns to 128
            for k in range(8):
                nc.sync.dma_start(idxB[16 * k:16 * (k + 1), :], idxB_t[:, :])
                nc.sync.dma_start(idxD[16 * k:16 * (k + 1), :], idxD_t[:, :])
            nc.vector.memset(zrow[:, :], 0.0)
            nc.sync.dma_start(e2_table[P["ZROW"]:P["ZROW"] + 1, :], zrow[:, :])

            # ---- dequantize x shard into the f16 stage, then AllGather
            # (stage: collectives can't read IO tensors directly)
            if "X" in KPH:
                for t in range(VT):
                    sl0 = slice(t * 128, (t + 1) * 128)
                    xq = wpool.tile([128, C + 2], u8, tag="xq")
                    nc.sync.dma_start(xq[:, :], xs_t[sl0, :])
                    xscf = wpool.tile([128, 1], f32, tag="xscf")
                    nc.scalar.copy(xscf[:, :], xq[:, C:C + 2].bitcast(f16))
                    xd = wpool.tile([128, C], f16, tag="xd")
                    nc.scalar.activation(xd[:, :], xq[:, 0:C].bitcast(i8),
                                         Ident, bias=0.0,
                                         scale=xscf[:, 0:1])
                    nc.sync.dma_start(x_stage[sl0, :], xd[:, :])
                nc.gpsimd.collective_compute(
                    "AllGather", mybir.AluOpType.bypass,
                    replica_groups=[list(range(NCORES))],
                    ins=[x_stage.ap().opt()],
                    outs=[x_full[0:GV_TOTAL, :].opt()])

            callB_of_tile = {}
            for (t, s, L, co) in P["callsB"]:
                callB_of_tile.setdefault(t, []).append((s, L, co))

            # ---- phase B per edge tile
            for t in (range(ET) if "B" in KPH else []):
                Lt = int(Llo[t] + Lhi[t])
                strip = spool.tile([128, Lt, C], f16, tag="strip")
                loff = 0
                for (s, L, co) in callB_of_tile[t]:
                    src = x_full[0:LO, :] if s == 0 else x_full[LO:GV_TOTAL, :]
                    nc.gpsimd.dma_gather(
                        strip[:, loff:loff + L, :], src,
                        idxB[:, co:co + L * 8], L * 128, L * 128, C,
                        single_packet=False)
                    loff += L
                sl = slice(t * 128, (t + 1) * 128)
                xsum = wpool.tile([128, C], f32, tag="xsum")
                nc.vector.tensor_reduce(
                    xsum[:, :], strip[:, :, :].rearrange("p l f -> p f l"),
                    AX, AddOp)
                xm = wpool.tile([128, C], f32, tag="xm")
                nc.scalar.activation(xm[:, :], xsum[:, :], Copy,
                                     bias=0.0, scale=re[:, t:t + 1])
                # transpose -> [feat, slot]
                pT = psA.tile([128, C], f32, tag="pT")
                nc.tensor.transpose(pT[:, :], xm[:, :], eye32[:, :])
                xmT = wpool.tile([128, C], f32, tag="xmT")
                nc.scalar.copy(xmT[:, :], pT[:, :])
                # W1 -> relu(+b1)
                pe = psB.tile([128, C], f32, tag="pe")
                nc.tensor.matmul(pe[:, :], w1t[:, :], xmT[:, :])
                eT = wpool.tile([128, C], f32, tag="eT")
                nc.scalar.activation(eT[:, :], pe[:, :], Relu,
                                     bias=b1[:, :], scale=1.0)
                # W2 -> +b2 (fp16)
                pe2 = psB.tile([128, C], f32, tag="pe2")
                nc.tensor.matmul(pe2[:, :], w2t[:, :], eT[:, :])
                e2T = wpool.tile([128, C], f16, tag="e2T")
                nc.scalar.activation(e2T[:, :], pe2[:, :], Ident,
                                     bias=b2[:, :], scale=1.0)
                # transpose back -> e2 rows, store shard
                pr = psA.tile([128, C], f16, tag="pr")
                nc.tensor.transpose(pr[:, :], e2T[:, :], eyeH[:, :])
                e2r = wpool.tile([128, C], f16, tag="e2r")
                nc.scalar.copy(e2r[:, :], pr[:, :])
                nc.sync.dma_start(e2_shard[sl, :], e2r[:, :])

            # ---- AllGather e2 shards
            if "C" in KPH:
                nc.gpsimd.collective_compute(
                    "AllGather", mybir.AluOpType.bypass,
                    replica_groups=[list(range(NCORES))],
                    ins=[e2_shard.ap().opt()],
                    outs=[e2_table[0:NCORES * ESLOTS, :].opt()])

            # ---- phase D
            for (t0, nt, Ls, co) in (P["callsD"] if "D" in KPH else []):
                dstrip = spool.tile([128, Ls, C], f16, tag="dstrip")
                nc.gpsimd.dma_gather(
                    dstrip[:, :, :], e2_table[:, :],
                    idxD[:, co:co + Ls * 8], Ls * 128, Ls * 128, C,
                    single_packet=False)
                for t in range(t0, t0 + nt):
                    L = int(Lv[t])
                    toff = int(P["tile_off"][t])
                    ysum = wpool.tile([128, C], f32, tag="ysum")
                    nc.vector.tensor_reduce(
                        ysum[:, :],
                        dstrip[:, toff:toff + L, :].rearrange("p l f -> p f l"),
                        AX, AddOp)
                    yt = wpool.tile([128, C], f32, tag="yt")
                    nc.scalar.activation(yt[:, :], ysum[:, :], Relu,
                                         bias=0.0, scale=rv[:, t:t + 1])
                    # per-row uint8 quantization: q = round(y * 254/rowmax)
                    rmax = wpool.tile([128, 1], f32, tag="rmax")
                    nc.vector.tensor_reduce(rmax[:, :], yt[:, :], AX, MaxOp)
                    # rmax <- max(rmax, eps) / 254   (the stored scale)
                    nc.vector.tensor_scalar(rmax[:, :], rmax[:, :], 1e-20,
                                            1.0 / 254.0, MaxOp, MultOp)
                    sinv = wpool.tile([128, 1], f32, tag="sinv")
                    nc.vector.reciprocal(sinv[:, :], rmax[:, :])
                    yq = wpool.tile([128, C], u8, tag="yq")
                    nc.scalar.activation(yq[:, :], yt[:, :], Ident,
                                         bias=half[:, :], scale=sinv[:, 0:1])
                    ysc = wpool.tile([128, 1], f16, tag="ysc")
                    nc.scalar.copy(ysc[:, :], rmax[:, :])
                    sl2 = slice(t * 128, (t + 1) * 128)
                    nc.sync.dma_start(y_t[sl2, 0:C], yq[:, :])
                    nc.sync.dma_start(y_t[sl2, C:C + 2],
                                      ysc[:, :].bitcast(u8))
    nc.compile()
    return nc


def _get_runner(nc):
    import jax
    import jax.numpy as jnp
    import concourse.mybir as mybir
    from concourse.bass2jax import (_bass_exec_p, install_neuronx_cc_hook,
                                    partition_id_tensor)
    from jax.sharding import Mesh, PartitionSpec, NamedSharding
    from jax.experimental.shard_map import shard_map

    install_neuronx_cc_hook()
    partition_name = (nc.partition_id_tensor.name
                      if nc.partition_id_tensor else None)
    in_names, out_names, out_avals = [], [], []
    for alloc in nc.m.functions[0].allocations:
        if not isinstance(alloc, mybir.MemoryLocationSet):
            continue
        name = alloc.memorylocations[0].name
        if alloc.kind == "ExternalInput":
            if name != partition_name:
                in_names.append(name)
        elif alloc.kind == "ExternalOutput":
            out_names.append(name)
            out_avals.append(jax.core.ShapedArray(
                tuple(alloc.tensor_shape), mybir.dt.np(alloc.dtype)))
    n_params = len(in_names)
    n_outs = len(out_names)
    all_names = in_names + out_names + (
        [partition_name] if partition_name else [])

    def _body(*args):
        operands = list(args)
        if partition_name is not None:
            operands.append(partition_id_tensor())
        outs = _bass_exec_p.bind(
            *operands, out_avals=tuple(out_avals),
            in_names=tuple(all_names), out_names=tuple(out_names),
            lowering_input_output_aliases=(), sim_require_finite=True,
            sim_require_nnan=True, nc=nc)
        return tuple(outs)

    devices = jax.devices()[:NCORES]
    mesh = Mesh(np.asarray(devices), ("core",))
    spec = PartitionSpec("core")
    in_specs = (spec,) * (n_params + n_outs)
    out_specs = (spec,) * n_outs
    donate = tuple(range(n_params, n_params + n_outs))
    fn = jax.jit(
        shard_map(_body, mesh=mesh, in_specs=in_specs,
                  out_specs=out_specs, check_rep=False),
        donate_argnums=donate, keep_unused=True)
    sh = NamedSharding(mesh, spec)
    zfns = [jax.jit(
        lambda a=av: jnp.zeros((NCORES * a.shape[0],) + a.shape[1:], a.dtype),
        out_shardings=sh) for av in out_avals]
    return dict(fn=fn, in_names=in_names, out_names=out_names,
                sh=sh, zfns=zfns, devices=devices)


def kernel(x, hyperedge_index, W_v2e, b_v2e, W_e2v, b_e2v):
    import jax
    import time
    KTIME = os.environ.get("HNHN_DEBUG_TIME", "0") == "1"
    tick = time.time

    t0 = tick()
    hb = np.asarray(hyperedge_index)
    cached_hb = _cache.get("hb")
    if (cached_hb is None or cached_hb.shape != hb.shape
            or cached_hb.dtype != hb.dtype or not np.array_equal(cached_hb, hb)):
        _cache.clear()
        _cache["hb"] = hb.copy()
        _cache["P"] = _prep(hb)
        _cache["nc"] = _build(_cache["P"])
        _cache["R"] = _get_runner(_cache["nc"])
        _cache["dev"] = {}
    P, R = _cache["P"], _cache["R"]
    dev = _cache["dev"]
    sh = R["sh"]
    pool = _cache.get("pool")
    if pool is None:
        from concurrent.futures import ThreadPoolExecutor
        pool = _cache["pool"] = ThreadPoolExecutor(NCORES)

    # per-row int8 quantization (f16 scale packed in the last 2 columns),
    # threaded across row chunks, then one async device_put
    txs = tick()
    xs = _cache.get("xs")
    if xs is None:
        xs = _cache["xs"] = np.zeros((GV_TOTAL, C + 2), np.uint8)
    xf = np.asarray(x, np.float32)
    npc = N_NODES // NCORES

    def _quant(c):
        xc = xf[c * npc:(c + 1) * npc]
        sc = (np.maximum(np.abs(xc).max(axis=1), 1e-20) / 127.0
              ).astype(np.float16)
        tmp = xc * (1.0 / sc.astype(np.float32))[:, None]
        np.rint(tmp, out=tmp)
        np.clip(tmp, -127, 127, out=tmp)
        dst = xs[c * VSLOTS:c * VSLOTS + npc]
        dst[:, 0:C] = tmp.astype(np.int8).view(np.uint8)
        dst[:, C:C + 2] = sc.view(np.uint8).reshape(-1, 2)
    list(pool.map(_quant, range(NCORES)))
    if KTIME: print("  xs scatter:", tick() - txs)
    dx = jax.device_put(xs, sh)
    if KTIME:
        _tu = tick(); jax.block_until_ready(dx)
        print("  x upload wait:", tick() - _tu)

    def put(name, arr):
        cur = dev.get(name)
        if cur is None or not (cur[0] is arr or np.array_equal(cur[0], arr)):
            dev[name] = (arr, jax.device_put(arr, sh))
        return dev[name][1]

    if "const_np" not in _cache:
        CB, CD = P["CB"], P["CD"]
        _cache["const_np"] = {
            "idxB": np.ascontiguousarray(P["idxB"].reshape(NCORES * 16, CB)),
            "idxD": np.ascontiguousarray(P["idxD"].reshape(NCORES * 16, CD)),
            "recip_e": np.ascontiguousarray(
                P["recip_e"].reshape(NCORES, ET, 128).transpose(0, 2, 1)
            ).reshape(NCORES * 128, ET),
            "recip_v": np.ascontiguousarray(
                P["recip_v"].reshape(NCORES, VT, 128).transpose(0, 2, 1)
            ).reshape(NCORES * 128, VT),
            "eye32": np.tile(np.eye(C, dtype=np.float32), (NCORES, 1)),
            "eyeH": np.tile(np.eye(C, dtype=np.float16), (NCORES, 1)),
        }
    cn = _cache["const_np"]

    # weights: compare the small untiled arrays, cache tiled device copies
    def putw(name, arr):
        cur = dev.get(name)
        if cur is None or not (cur[0] is arr or np.array_equal(cur[0], arr)):
            tiled = np.tile(np.ascontiguousarray(arr), (NCORES, 1))
            dev[name] = (arr, jax.device_put(tiled, sh))
        return dev[name][1]

    w1t = np.asarray(W_v2e, np.float32).T
    w2t = np.asarray(W_e2v, np.float32).T
    b1 = np.asarray(b_v2e, np.float32).reshape(C, 1)
    b2 = np.asarray(b_e2v, np.float32).reshape(C, 1)

    named = {"idxB": cn["idxB"], "idxD": cn["idxD"],
             "recip_e": cn["recip_e"], "recip_v": cn["recip_v"],
             "eye32": cn["eye32"], "eyeH": cn["eyeH"]}
    wnamed = {"w1t": w1t, "w2t": w2t, "b1": b1, "b2": b2}
    args = []
    for name in R["in_names"]:
        if name == "x_shard":
            args.append(dx)
        elif name in wnamed:
            args.append(putw(name, wnamed[name]))
        else:
            args.append(put(name, named[name]))
    # donate the previous call's output buffer when available (the kernel
    # writes every row of y, so initial contents are irrelevant)
    zeros = _cache.pop("donate_next", None)
    if zeros is None:
        zeros = [zf() for zf in R["zfns"]]
    if KTIME:
        jax.block_until_ready(args); jax.block_until_ready(zeros)
        print("  consts+zeros+xwait:", tick() - t0)
        t0 = tick()
    outs = R["fn"](*args, *zeros)
    if KTIME:
        jax.block_until_ready(outs)
        print("  exec:", tick() - t0)
        t0 = tick()
    yi = R["out_names"].index("y")
    out = np.empty((N_NODES, C), np.float32)
    if "inv" not in _cache:
        # per-core: node ids owned by that core (ordered) and their slots
        g_v = P["g_v"]
        core = g_v // VSLOTS
        _cache["inv"] = [(np.flatnonzero(core == c),
                          g_v[core == c] - c * VSLOTS)
                         for c in range(NCORES)]
    inv = _cache["inv"]
    shards = outs[yi].addressable_shards

    def _fetch(s):
        c = s.index[0].start // VSLOTS
        ys = np.asarray(s.data)  # [VSLOTS, C+2] uint8
        nodes, slots = inv[c]
        sc = ys[slots, C:C + 2].copy().view(np.float16).astype(np.float32)
        out[nodes] = ys[slots, 0:C] * sc
    list(pool.map(_fetch, shards))
    _cache["donate_next"] = list(outs)
    if KTIME: print("  y fetch+scatter:", tick() - t0)
    return out
